# revision 27
# baseline (speedup 1.0000x reference)
"""AutoDisBucketEncoder Trainium2 kernel (8 NeuronCores, feature-sharded).

Math (per feature f, batch b):
  h = lrelu(x_aug @ w1_aug)            # bias folded via ones column
  h = lrelu(h @ (rw_l + I) + rb_l)     # x3, residual folded into weights
  z = lrelu(h @ w2 + b2)
  e = exp(z * tau)
  out = (e / sum_k e) @ emb

Layout: features sharded 32/core; each core packs 2 features per 128
partitions (block-diagonal weights), streams the full 2048 batch as the
matmul moving dim.  Softmax runs in [k, b] layout; the sum-over-k and its
broadcast back to 128 partitions are done by one ones-block matmul; the
normalize is one fused custom DVE op (mul by 1-NR reciprocal).  Step-0
matmuls are row-tiled 4-ways (contraction 8).  The embedding matmul uses
normalized probabilities as the stationary operand so its PSUM output
lands directly in [batch, emb] layout; emb matmuls are emitted j-pairwise
interleaved so LDWEIGHTS hides under the other row-group's matmul.
All activations use Prelu (parametric_relu) so Exp and the relu share one
ACT table set (no ACT_TABLE_LOAD swaps).
"""

import sys

sys.path.insert(0, "/opt/trn_rl_repo")

import numpy as np
import ml_dtypes
from contextlib import ExitStack

BF16 = ml_dtypes.bfloat16
B, F, D, K, E = 2048, 256, 64, 8, 128
NCORES = 8
FC = F // NCORES          # 32 features per core
NPAIR = FC // 2           # 16
NSTACK = NPAIR // 4       # 4 stacks of 4 pairs
NEG = 0.01                # leaky slope
HB = B // 2               # 1024 batch half-chunk (2 PSUM banks in f32)

# h-chunk evictions: idx % MOD < LIM -> DVE custom-op path, else ACT op
DVE_RES_MOD = 2
DVE_RES_LIM = 1

_compiled = None
SIM_SAFE = False  # substitute Relu for Prelu so CoreSim can execute
N_FILL = 10  # LDWEIGHTS fillers per step boundary (HAM keep-warm)


def _register_leaky_bias():
    import numpy as np
    from concourse.dve_spec import Spec, Src0, C0, C1, maxx, lower
    from concourse.dve_ops import (
        DveOp, DveOpSpec, OPS, CUSTOM_DVE_SPECS, _SUB_OPCODE_FOR_NAME,
        _CUSTOM_DVE_ROW_BASE, has_src1,
    )

    if "LEAKY_BIAS_ANT" in CUSTOM_DVE_SPECS:
        return next(o for o in OPS if o.name == "LEAKY_BIAS_ANT")
    spec = Spec(
        body=maxx(Src0 + C0, (Src0 + C0) * C1),
        reference=lambda in0, in1, s0, s1, imm2: np.maximum(
            in0 + s0, (in0 + s0) * s1
        ).astype(np.float32),
    )
    row = _CUSTOM_DVE_ROW_BASE + len(OPS)
    shas = {}
    for ver in ("v3", "v4"):
        uops = lower(spec, ver=ver)
        shas[ver] = DveOpSpec(
            name="LEAKY_BIAS_ANT", opcode=row, uops=uops, rd1_en=has_src1(spec)
        ).sha(ver)
    op = DveOp("LEAKY_BIAS_ANT", spec, subdim=False, uops_sha=shas)
    OPS.append(op)
    CUSTOM_DVE_SPECS[op.name] = spec
    _SUB_OPCODE_FOR_NAME[op.name] = row
    return op


def _register_softmax_norm():
    """out = Src1 * approx_recip(Src0), seed + one Newton step (~0.4% rel).

    Src0 = softmax denominator (fp32, PSUM), Src1 = exp values (bf16).
    Folds reciprocal + cast + multiply into one DVE pass.
    """
    import numpy as np
    from concourse.dve_spec import Spec, Src0, Src1, C0, C1, Bin, AluOp, lower
    from concourse.dve_ops import (
        DveOp, DveOpSpec, OPS, CUSTOM_DVE_SPECS, _SUB_OPCODE_FOR_NAME,
        _CUSTOM_DVE_ROW_BASE, has_src1,
    )

    if "SOFTMAX_NORM_ANT" in CUSTOM_DVE_SPECS:
        return next(o for o in OPS if o.name == "SOFTMAX_NORM_ANT")

    _not_x = Bin(AluOp.BITWISE_NOT, Src0, Src0)
    _y0 = _not_x * C0
    _y1 = _y0 * (C1 - Src0 * _y0)

    def _ref(in0, in1, s0, s1, imm2):
        not_x = (~in0.astype(np.float32).view(np.int32)).view(np.float32)
        y0 = not_x * s0
        y1 = y0 * (s1 - in0 * y0)
        return (in1.astype(np.float32) * y1).astype(np.float32)

    spec = Spec(body=Src1 * _y1, reference=_ref)
    row = _CUSTOM_DVE_ROW_BASE + len(OPS)
    shas = {}
    for ver in ("v3", "v4"):
        uops = lower(spec, ver=ver)
        shas[ver] = DveOpSpec(
            name="SOFTMAX_NORM_ANT", opcode=row, uops=uops, rd1_en=has_src1(spec)
        ).sha(ver)
    op = DveOp("SOFTMAX_NORM_ANT", spec, subdim=False, uops_sha=shas)
    OPS.append(op)
    CUSTOM_DVE_SPECS[op.name] = spec
    _SUB_OPCODE_FOR_NAME[op.name] = row
    return op


# Chebyshev seed constants shared with RECIPROCAL_APPROX_FAST; one NR pass.
NORM_C0 = -0.23549792
NORM_C1 = 2.0017324


def _build_bass():
    import concourse.bass as bass  # noqa: F401
    import concourse.mybir as mybir
    import concourse.tile as tile
    from concourse import bacc

    LEAKY_OP = _register_leaky_bias()
    NORM_OP = _register_softmax_norm()

    dt = mybir.dt
    AF = mybir.ActivationFunctionType
    LRELU = AF.Relu if SIM_SAFE else AF.Prelu

    nc = bacc.Bacc("TRN2", target_bir_lowering=False, debug=False)

    xp = nc.dram_tensor("xp", [NSTACK, 128, B], dt.bfloat16, kind="ExternalInput").ap()
    w1p = nc.dram_tensor("w1p", [128, NSTACK * 128], dt.bfloat16, kind="ExternalInput").ap()
    rwp = nc.dram_tensor("rwp", [128, 3 * NPAIR * 128], dt.bfloat16, kind="ExternalInput").ap()
    rbp = nc.dram_tensor("rbp", [128, 3 * NPAIR], dt.float32, kind="ExternalInput").ap()
    w2p = nc.dram_tensor("w2p", [128, NPAIR * 32], dt.bfloat16, kind="ExternalInput").ap()
    b2s = nc.dram_tensor("b2s", [128, NSTACK], dt.float32, kind="ExternalInput").ap()
    taus = nc.dram_tensor("taus", [128, NSTACK], dt.float32, kind="ExternalInput").ap()
    onesbd = nc.dram_tensor("onesbd", [128, 128], dt.bfloat16, kind="ExternalInput").ap()
    embs = nc.dram_tensor("embs", [128, NSTACK * 256], dt.bfloat16, kind="ExternalInput").ap()
    out = nc.dram_tensor("out", [B, FC * E], dt.bfloat16, kind="ExternalOutput").ap()

    with tile.TileContext(nc) as tc, ExitStack() as ctx:
        const = ctx.enter_context(tc.tile_pool(name="const", bufs=1))
        xpool = ctx.enter_context(tc.tile_pool(name="xpool", bufs=3))
        hpool = ctx.enter_context(tc.tile_pool(name="hpool", bufs=8))
        tpool = ctx.enter_context(tc.tile_pool(name="tpool", bufs=2))
        epool = ctx.enter_context(tc.tile_pool(name="epool", bufs=2))
        opool = ctx.enter_context(tc.tile_pool(name="opool", bufs=4))
        h_ps = ctx.enter_context(tc.tile_pool(name="h_ps", bufs=3, space="PSUM"))
        zs_ps = ctx.enter_context(tc.tile_pool(name="zs_ps", bufs=1, space="PSUM"))

        # ---- constants into SBUF (all host-side pre-transposed, contiguous).
        # Inputs ride the gpsimd DMA queue so they never wait behind the
        # output DMAs (which block on eviction sems on the sync queue). ----
        w1_sb = const.tile([128, NSTACK * 128], dt.bfloat16)
        nc.gpsimd.dma_start(out=w1_sb, in_=w1p)
        # first x chunk right behind w1 so chunk 0 starts ~immediately
        xpool_first = xpool.tile([128, HB], dt.bfloat16, tag="x", name="x0_0")
        nc.gpsimd.dma_start(out=xpool_first, in_=xp[0][:, 0:HB])
        rb_sb = const.tile([128, 3 * NPAIR], dt.float32)
        nc.gpsimd.dma_start(out=rb_sb, in_=rbp)
        rw_sb = const.tile([128, 3 * NPAIR * 128], dt.bfloat16)
        for l in range(3):
            nc.gpsimd.dma_start(
                out=rw_sb[:, l * NPAIR * 128 : (l + 1) * NPAIR * 128],
                in_=rwp[:, l * NPAIR * 128 : (l + 1) * NPAIR * 128],
            )
        w2_sb = const.tile([128, NPAIR * 32], dt.bfloat16)
        nc.gpsimd.dma_start(out=w2_sb, in_=w2p)
        b2_sb = const.tile([128, NSTACK], dt.float32)
        nc.gpsimd.dma_start(out=b2_sb, in_=b2s)
        tau_sb = const.tile([128, NSTACK], dt.float32)
        nc.gpsimd.dma_start(out=tau_sb, in_=taus)
        ones_sb = const.tile([128, 128], dt.bfloat16)
        nc.gpsimd.dma_start(out=ones_sb, in_=onesbd)
        emb_sb = const.tile([128, NSTACK * 256], dt.bfloat16)
        nc.gpsimd.dma_start(out=emb_sb, in_=embs)

        # out[b, fc*E] viewed as [qb(4), s(4), p(128), i(4), j(4), e(256)]
        out_r = out.rearrange("(qb i p) (s j e) -> qb s p i j e", p=128, i=4, j=4, e=256)

        def evict_h(idx, h, ph, rb_ap):
            """psum -> sbuf bf16 with (optional bias add and) leaky relu."""
            if idx % DVE_RES_MOD < DVE_RES_LIM:
                nc.vector._custom_dve(
                    LEAKY_OP,
                    out=h,
                    in0=ph,
                    s0=0.0 if rb_ap is None else rb_ap,
                    s1=NEG,
                )
            else:
                if rb_ap is None:
                    nc.scalar.activation(h, ph, LRELU, alpha=NEG)
                else:
                    nc.scalar.activation(h, ph, LRELU, bias=rb_ap, alpha=NEG)

        res_idx = 0
        po_idx = 0
        pending_zsum = []  # deferred sum-over-k matmuls
        pending_tail = []  # deferred fused softmax-normalize
        pending_embmm = []  # emb matmul jpair closures
        pending_poev = []   # po eviction closures (lag one slot behind MMs)

        def emit(lst, n=99):
            for _ in range(min(n, len(lst))):
                lst.pop(0)()

        # prefetch x one chunk ahead on the gpsimd DMA queue
        chunks = [(s, c) for s in range(NSTACK) for c in range(2)]
        x_tiles = {}

        def fetch_x(k):
            if k >= len(chunks):
                return
            s_, c_ = chunks[k]
            t = xpool.tile([128, HB], dt.bfloat16, tag="x", name=f"x{s_}_{c_}")
            nc.gpsimd.dma_start(out=t, in_=xp[s_][:, c_ * HB : (c_ + 1) * HB])
            x_tiles[k] = t

        x_tiles[0] = xpool_first
        for s in range(NSTACK):
            e_sb = epool.tile([128, B], dt.bfloat16, tag="e", name=f"e{s}")
            en_sb = epool.tile([128, B], dt.bfloat16, tag="en", name=f"en{s}")
            for c in range(2):
                # ---- h pipeline: pairs interleaved layer-step-wise; the
                # previous chunk's softmax/emb work drips in between steps ----
                ck = s * 2 + c
                fetch_x(ck + 1)
                x_sb = x_tiles.pop(ck)
                hs = [x_sb] * 4
                for step in range(4):  # 0: L1 (row-tiled), 1-3: residual layers
                    for j in range(4):
                        p = 4 * s + j
                        ph = h_ps.tile(
                            [128, HB], dt.float32, tag="h", name=f"ph{p}_{c}_{step}"
                        )
                        if step == 0:
                            wsl = w1_sb[32 * j : 32 * j + 8, s * 128 : (s + 1) * 128]
                            rb_ap = None
                            for q in range(2):
                                nc.tensor.matmul(
                                    ph[:, q * 512 : (q + 1) * 512],
                                    wsl,
                                    hs[j][32 * j : 32 * j + 8, q * 512 : (q + 1) * 512],
                                    start=True,
                                    stop=True,
                                    tile_position=(32 * j, 0),
                                )
                        else:
                            l = step - 1
                            wsl = rw_sb[
                                :, (l * NPAIR + p) * 128 : (l * NPAIR + p + 1) * 128
                            ]
                            rb_ap = rb_sb[:, l * NPAIR + p : l * NPAIR + p + 1]
                            for q in range(2):
                                nc.tensor.matmul(
                                    ph[:, q * 512 : (q + 1) * 512],
                                    wsl,
                                    hs[j][:, q * 512 : (q + 1) * 512],
                                    start=True,
                                    stop=True,
                                )
                        h2 = hpool.tile(
                            [128, HB], dt.bfloat16, tag="h", name=f"h{p}_{c}_{step}"
                        )
                        evict_h(res_idx, h2, ph, rb_ap)
                        res_idx += 1
                        hs[j] = h2
                    # keep the PE array active across the eviction-paced gap
                    # so the HAM clock gate stays at full rate (LDWEIGHTS has
                    # no deps and touches no PSUM)
                    for _ in range(N_FILL):
                        nc.tensor.ldweights(weights=rw_sb[:, 0:128])
                    if step == 1:
                        emit(pending_zsum)
                    elif step == 2:
                        emit(pending_tail)
                    elif step == 3:
                        emit(pending_embmm, 1)
                pz = zs_ps.tile([128, HB], dt.float32, tag="zs", name=f"pz{s}_{c}")
                for j in range(4):
                    p = 4 * s + j
                    for q in range(2):
                        nc.tensor.matmul(
                            pz[32 * j : 32 * j + 32, q * 512 : (q + 1) * 512],
                            w2_sb[:, p * 32 : (p + 1) * 32],
                            hs[j][:, q * 512 : (q + 1) * 512],
                            start=True,
                            stop=True,
                            tile_position=(0, 32 * j),
                        )
                # interleave the previous chunk's emb MMs and (lagged) po
                # evictions so the ACT/DVE queues never park on an eviction
                # whose matmuls haven't run yet.
                while pending_embmm or pending_poev:
                    emit(pending_poev, 1)
                    if pending_embmm:
                        emit(pending_embmm, 1)

                def make_z(s_, c_, pz_ref, e_ref, en_ref):
                    def z_chain():
                        t1 = tpool.tile(
                            [128, HB], dt.float32, tag="zt", name=f"t1_{s_}_{c_}"
                        )
                        nc.scalar.activation(
                            t1, pz_ref, LRELU,
                            bias=b2_sb[:, s_ : s_ + 1], alpha=NEG,
                        )
                        ev = e_ref[:, c_ * HB : (c_ + 1) * HB]
                        nc.scalar.activation(
                            ev, t1, AF.Exp, scale=tau_sb[:, s_ : s_ + 1]
                        )

                        def zsum():
                            ps_sum = zs_ps.tile(
                                [128, HB], dt.float32, tag="zs", name=f"psum{s_}_{c_}"
                            )
                            for q in range(2):
                                nc.tensor.matmul(
                                    ps_sum[:, q * 512 : (q + 1) * 512],
                                    ones_sb,
                                    ev[:, q * 512 : (q + 1) * 512],
                                    start=True,
                                    stop=True,
                                )

                            def tail():
                                nc.vector._custom_dve(
                                    NORM_OP,
                                    out=en_ref[:, c_ * HB : (c_ + 1) * HB],
                                    in0=ps_sum,
                                    in1=ev,
                                    s0=NORM_C0,
                                    s1=NORM_C1,
                                )

                            pending_tail.append(tail)

                        pending_zsum.append(zsum)

                    return z_chain

                # this chunk's z epilogue (t1+exp) is emitted right away:
                # the ACT queue reaches it well after the pz matmuls drain.
                make_z(s, c, pz, e_sb, en_sb)()

                def make_jpair(s_, qb_, jp_, en_ref, ob_ref):
                    # two emb row-groups (j = 2*jp_, 2*jp_+1) interleaved
                    # i-major so each LDWEIGHTS hides under the other
                    # group's matmul; evictions lag one jpair behind the
                    # matmuls so ACT/DVE queues never park on them.
                    def emit_jpair_mms():
                        js = (2 * jp_, 2 * jp_ + 1)
                        pos = {}
                        for j_ in js:
                            pos[j_] = h_ps.tile(
                                [128, 4, 256],
                                dt.float32,
                                tag="h",
                                name=f"po{s_}_{qb_}_{j_}",
                            )
                        for i in range(4):
                            bc2 = qb_ * 4 + i
                            for j_ in js:
                                nc.tensor.matmul(
                                    pos[j_][:, i, :],
                                    en_ref[
                                        32 * j_ : 32 * j_ + 16,
                                        bc2 * 128 : (bc2 + 1) * 128,
                                    ],
                                    emb_sb[
                                        32 * j_ : 32 * j_ + 16,
                                        s_ * 256 : (s_ + 1) * 256,
                                    ],
                                    start=True,
                                    stop=True,
                                    tile_position=(32 * j_, 0),
                                )

                        def evict_pair():
                            nonlocal po_idx
                            for j_ in js:
                                if po_idx % 2 == 0:
                                    nc.vector.tensor_copy(ob_ref[:, :, j_, :], pos[j_])
                                else:
                                    nc.scalar.activation(
                                        ob_ref[:, :, j_, :], pos[j_], AF.Copy
                                    )
                                po_idx += 1
                            if jp_ == 1:
                                nc.sync.dma_start(out=out_r[qb_, s_], in_=ob_ref)

                        pending_poev.append(evict_pair)

                    return emit_jpair_mms

                for qb in (2 * c, 2 * c + 1):
                    ob = opool.tile(
                        [128, 4, 4, 256], dt.bfloat16, tag="o", name=f"ob{s}_{qb}"
                    )
                    for jp in range(2):
                        pending_embmm.append(make_jpair(s, qb, jp, en_sb, ob))
        for _ in range(6):
            nc.tensor.ldweights(weights=rw_sb[:, 0:128])
        emit(pending_zsum)
        for _ in range(6):
            nc.tensor.ldweights(weights=rw_sb[:, 0:128])
        emit(pending_tail)
        while pending_embmm or pending_poev:
            if pending_embmm:
                emit(pending_embmm, 1)
            emit(pending_poev, 1)

    nc.compile()
    return nc


def _host_pack(inputs):
    """Pack full f32 inputs into per-core bf16 device arrays."""
    x = np.ascontiguousarray(inputs["x"], dtype=np.float32)
    w1 = np.asarray(inputs["w1"], dtype=np.float32)
    b1 = np.asarray(inputs["b1"], dtype=np.float32)
    w2 = np.asarray(inputs["w2"], dtype=np.float32)
    b2 = np.asarray(inputs["b2"], dtype=np.float32)
    tau = np.asarray(inputs["tau"], dtype=np.float32)
    emb = np.asarray(inputs["emb"], dtype=np.float32)
    rws = [np.asarray(inputs[f"rw{l}"], dtype=np.float32) for l in range(3)]
    rbs = [np.asarray(inputs[f"rb{l}"], dtype=np.float32) for l in range(3)]

    eye = np.eye(D, dtype=np.float32)
    xT = np.concatenate([x, np.ones((B, F, 1), np.float32)], axis=2)
    xT = np.ascontiguousarray(xT.transpose(1, 2, 0))  # [F, 4, B]
    w1a = np.concatenate([w1, b1[:, None, :]], axis=1)  # [F, 4, D]

    in_maps = []
    for cidx in range(NCORES):
        f0 = cidx * FC
        xpk = np.zeros((NSTACK, 128, B), BF16)
        w1k = np.zeros((128, NSTACK, 128), BF16)
        rwk = np.zeros((128, 3, NPAIR, 128), BF16)
        rbk = np.zeros((128, 3, NPAIR), np.float32)
        w2k = np.zeros((128, NPAIR, 32), BF16)
        b2k = np.zeros((128, NSTACK), np.float32)
        tauk = np.zeros((128, NSTACK), np.float32)
        # garbage partitions keep tau=0 so exp(0)=1 stays finite
        embk = np.zeros((128, NSTACK, 256), BF16)
        for pr in range(NPAIR):
            fa, fb = f0 + 2 * pr, f0 + 2 * pr + 1
            s, jj = pr // 4, pr % 4
            xpk[s, 32 * jj : 32 * jj + 4] = xT[fa]
            xpk[s, 32 * jj + 4 : 32 * jj + 8] = xT[fb]
            w1k[32 * jj : 32 * jj + 4, s, 0:64] = w1a[fa]
            w1k[32 * jj + 4 : 32 * jj + 8, s, 64:128] = w1a[fb]
            for l in range(3):
                rwk[0:64, l, pr, 0:64] = rws[l][fa] + eye
                rwk[64:128, l, pr, 64:128] = rws[l][fb] + eye
                rbk[0:64, l, pr] = rbs[l][fa]
                rbk[64:128, l, pr] = rbs[l][fb]
            w2k[0:64, pr, 0:8] = w2[fa]
            w2k[64:128, pr, 8:16] = w2[fb]
            for fi, ff in ((0, fa), (1, fb)):
                rows = slice(32 * jj + 8 * fi, 32 * jj + 8 * fi + 8)
                b2k[rows, s] = b2[ff]
                tauk[rows, s] = tau[ff]
                embk[rows, s, 128 * fi : 128 * fi + 128] = emb[ff]
        # sum-over-k stationary with broadcast to all 128 rows; garbage
        # partitions duplicate the pair's second feature so values stay sane.
        ob = np.zeros((128, 128), BF16)
        for jj in range(4):
            for g in range(4):
                src = 32 * jj + 8 * min(g, 1)
                ob[src : src + 8, 32 * jj + 8 * g : 32 * jj + 8 * g + 8] = 1
        m = {
            "xp": xpk,
            "w1p": w1k.reshape(128, NSTACK * 128),
            "rwp": rwk.reshape(128, 3 * NPAIR * 128),
            "rbp": rbk.reshape(128, 3 * NPAIR),
            "w2p": w2k.reshape(128, NPAIR * 32),
            "b2s": b2k,
            "taus": tauk,
            "embs": embk.reshape(128, NSTACK * 256),
            "onesbd": ob,
        }
        in_maps.append(m)
    return in_maps


def _get_compiled():
    global _compiled
    if _compiled is None:
        _compiled = _build_bass()
    return _compiled


def run_on_hw(in_maps, trace=False):
    from concourse import bass_utils

    nc = _get_compiled()
    res = bass_utils.run_bass_kernel_spmd(
        nc, in_maps, core_ids=list(range(NCORES)), trace=trace
    )
    return res


def kernel(**inputs):
    in_maps = _host_pack(inputs)
    res = run_on_hw(in_maps, trace=False)
    outs = [np.asarray(res.results[c]["out"], dtype=np.float32) for c in range(NCORES)]
    return np.concatenate(outs, axis=1)


# revision 29
# speedup vs baseline: 1.0288x; 1.0288x over previous
"""AutoDisBucketEncoder Trainium2 kernel (8 NeuronCores, feature-sharded).

Math (per feature f, batch b):
  h = lrelu(x_aug @ w1_aug)            # bias folded via ones column
  h = lrelu(h @ (rw_l + I) + rb_l)     # x3, residual folded into weights
  z = lrelu(h @ w2 + b2)
  e = exp(z * tau)
  out = (e / sum_k e) @ emb

Layout: features sharded 32/core; each core packs 2 features per 128
partitions (block-diagonal weights), streams the full 2048 batch as the
matmul moving dim.  Softmax runs in [k, b] layout; the sum-over-k and its
broadcast back to 128 partitions are done by one ones-block matmul; the
normalize is one fused custom DVE op (mul by 1-NR reciprocal).  Step-0
matmuls are row-tiled 4-ways (contraction 8).  The embedding matmul uses
normalized probabilities as the stationary operand so its PSUM output
lands directly in [batch, emb] layout; emb matmuls are emitted j-pairwise
interleaved so LDWEIGHTS hides under the other row-group's matmul.
All activations use Prelu (parametric_relu) so Exp and the relu share one
ACT table set (no ACT_TABLE_LOAD swaps).
"""

import sys

sys.path.insert(0, "/opt/trn_rl_repo")

import numpy as np
import ml_dtypes
from contextlib import ExitStack

BF16 = ml_dtypes.bfloat16
B, F, D, K, E = 2048, 256, 64, 8, 128
NCORES = 8
FC = F // NCORES          # 32 features per core
NPAIR = FC // 2           # 16
NSTACK = NPAIR // 4       # 4 stacks of 4 pairs
NEG = 0.01                # leaky slope
HB = B // 2               # 1024 batch half-chunk (2 PSUM banks in f32)

# h-chunk evictions: idx % MOD < LIM -> DVE custom-op path, else ACT op
DVE_RES_MOD = 2
DVE_RES_LIM = 1

_compiled = None
SIM_SAFE = False  # substitute Relu for Prelu so CoreSim can execute
N_FILL = 6  # LDWEIGHTS fillers per step boundary (HAM keep-warm)


def _register_leaky_bias():
    import numpy as np
    from concourse.dve_spec import Spec, Src0, C0, C1, maxx, lower
    from concourse.dve_ops import (
        DveOp, DveOpSpec, OPS, CUSTOM_DVE_SPECS, _SUB_OPCODE_FOR_NAME,
        _CUSTOM_DVE_ROW_BASE, has_src1,
    )

    if "LEAKY_BIAS_ANT" in CUSTOM_DVE_SPECS:
        return next(o for o in OPS if o.name == "LEAKY_BIAS_ANT")
    spec = Spec(
        body=maxx(Src0 + C0, (Src0 + C0) * C1),
        reference=lambda in0, in1, s0, s1, imm2: np.maximum(
            in0 + s0, (in0 + s0) * s1
        ).astype(np.float32),
    )
    row = _CUSTOM_DVE_ROW_BASE + len(OPS)
    shas = {}
    for ver in ("v3", "v4"):
        uops = lower(spec, ver=ver)
        shas[ver] = DveOpSpec(
            name="LEAKY_BIAS_ANT", opcode=row, uops=uops, rd1_en=has_src1(spec)
        ).sha(ver)
    op = DveOp("LEAKY_BIAS_ANT", spec, subdim=False, uops_sha=shas)
    OPS.append(op)
    CUSTOM_DVE_SPECS[op.name] = spec
    _SUB_OPCODE_FOR_NAME[op.name] = row
    return op


def _register_softmax_norm():
    """out = Src1 * approx_recip(Src0), seed + one Newton step (~0.4% rel).

    Src0 = softmax denominator (fp32, PSUM), Src1 = exp values (bf16).
    Folds reciprocal + cast + multiply into one DVE pass.
    """
    import numpy as np
    from concourse.dve_spec import Spec, Src0, Src1, C0, C1, Bin, AluOp, lower
    from concourse.dve_ops import (
        DveOp, DveOpSpec, OPS, CUSTOM_DVE_SPECS, _SUB_OPCODE_FOR_NAME,
        _CUSTOM_DVE_ROW_BASE, has_src1,
    )

    if "SOFTMAX_NORM_ANT" in CUSTOM_DVE_SPECS:
        return next(o for o in OPS if o.name == "SOFTMAX_NORM_ANT")

    _not_x = Bin(AluOp.BITWISE_NOT, Src0, Src0)
    _y0 = _not_x * C0
    _y1 = _y0 * (C1 - Src0 * _y0)

    def _ref(in0, in1, s0, s1, imm2):
        not_x = (~in0.astype(np.float32).view(np.int32)).view(np.float32)
        y0 = not_x * s0
        y1 = y0 * (s1 - in0 * y0)
        return (in1.astype(np.float32) * y1).astype(np.float32)

    spec = Spec(body=Src1 * _y1, reference=_ref)
    row = _CUSTOM_DVE_ROW_BASE + len(OPS)
    shas = {}
    for ver in ("v3", "v4"):
        uops = lower(spec, ver=ver)
        shas[ver] = DveOpSpec(
            name="SOFTMAX_NORM_ANT", opcode=row, uops=uops, rd1_en=has_src1(spec)
        ).sha(ver)
    op = DveOp("SOFTMAX_NORM_ANT", spec, subdim=False, uops_sha=shas)
    OPS.append(op)
    CUSTOM_DVE_SPECS[op.name] = spec
    _SUB_OPCODE_FOR_NAME[op.name] = row
    return op


# Chebyshev seed constants shared with RECIPROCAL_APPROX_FAST; one NR pass.
NORM_C0 = -0.23549792
NORM_C1 = 2.0017324


def _build_bass():
    import concourse.bass as bass  # noqa: F401
    import concourse.mybir as mybir
    import concourse.tile as tile
    from concourse import bacc

    LEAKY_OP = _register_leaky_bias()
    NORM_OP = _register_softmax_norm()

    dt = mybir.dt
    AF = mybir.ActivationFunctionType
    LRELU = AF.Relu if SIM_SAFE else AF.Prelu

    nc = bacc.Bacc("TRN2", target_bir_lowering=False, debug=False)

    xp = nc.dram_tensor("xp", [NSTACK, 128, B], dt.bfloat16, kind="ExternalInput").ap()
    w1p = nc.dram_tensor("w1p", [128, NSTACK * 128], dt.bfloat16, kind="ExternalInput").ap()
    rwp = nc.dram_tensor("rwp", [128, 3 * NPAIR * 128], dt.bfloat16, kind="ExternalInput").ap()
    rbp = nc.dram_tensor("rbp", [128, 3 * NPAIR], dt.float32, kind="ExternalInput").ap()
    w2p = nc.dram_tensor("w2p", [128, NPAIR * 32], dt.bfloat16, kind="ExternalInput").ap()
    b2s = nc.dram_tensor("b2s", [128, NSTACK], dt.float32, kind="ExternalInput").ap()
    taus = nc.dram_tensor("taus", [128, NSTACK], dt.float32, kind="ExternalInput").ap()
    onesbd = nc.dram_tensor("onesbd", [128, 128], dt.bfloat16, kind="ExternalInput").ap()
    embs = nc.dram_tensor("embs", [128, NSTACK * 256], dt.bfloat16, kind="ExternalInput").ap()
    out = nc.dram_tensor("out", [B, FC * E], dt.bfloat16, kind="ExternalOutput").ap()

    with tile.TileContext(nc) as tc, ExitStack() as ctx:
        const = ctx.enter_context(tc.tile_pool(name="const", bufs=1))
        xpool = ctx.enter_context(tc.tile_pool(name="xpool", bufs=3))
        hpool = ctx.enter_context(tc.tile_pool(name="hpool", bufs=8))
        tpool = ctx.enter_context(tc.tile_pool(name="tpool", bufs=2))
        epool = ctx.enter_context(tc.tile_pool(name="epool", bufs=2))
        opool = ctx.enter_context(tc.tile_pool(name="opool", bufs=4))
        h_ps = ctx.enter_context(tc.tile_pool(name="h_ps", bufs=3, space="PSUM"))
        zs_ps = ctx.enter_context(tc.tile_pool(name="zs_ps", bufs=1, space="PSUM"))

        # ---- constants into SBUF (all host-side pre-transposed, contiguous).
        # Inputs ride the gpsimd DMA queue so they never wait behind the
        # output DMAs (which block on eviction sems on the sync queue). ----
        w1_sb = const.tile([128, NSTACK * 128], dt.bfloat16)
        nc.gpsimd.dma_start(out=w1_sb, in_=w1p)
        # first x chunk right behind w1 so chunk 0 starts ~immediately
        xpool_first = xpool.tile([128, HB], dt.bfloat16, tag="x", name="x0_0")
        nc.gpsimd.dma_start(out=xpool_first, in_=xp[0][:, 0:HB])
        rb_sb = const.tile([128, 3 * NPAIR], dt.float32)
        nc.gpsimd.dma_start(out=rb_sb, in_=rbp)
        rw_sb = const.tile([128, 3 * NPAIR * 128], dt.bfloat16)
        for l in range(3):
            nc.gpsimd.dma_start(
                out=rw_sb[:, l * NPAIR * 128 : (l + 1) * NPAIR * 128],
                in_=rwp[:, l * NPAIR * 128 : (l + 1) * NPAIR * 128],
            )
        w2_sb = const.tile([128, NPAIR * 32], dt.bfloat16)
        nc.gpsimd.dma_start(out=w2_sb, in_=w2p)
        b2_sb = const.tile([128, NSTACK], dt.float32)
        nc.gpsimd.dma_start(out=b2_sb, in_=b2s)
        tau_sb = const.tile([128, NSTACK], dt.float32)
        nc.gpsimd.dma_start(out=tau_sb, in_=taus)
        ones_sb = const.tile([128, 128], dt.bfloat16)
        nc.gpsimd.dma_start(out=ones_sb, in_=onesbd)
        emb_sb = const.tile([128, NSTACK * 256], dt.bfloat16)
        nc.gpsimd.dma_start(out=emb_sb, in_=embs)

        # out[b, fc*E] viewed as [qb(4), s(4), p(128), i(4), j(4), e(256)]
        out_r = out.rearrange("(qb i p) (s j e) -> qb s p i j e", p=128, i=4, j=4, e=256)

        def evict_h(idx, h, ph, rb_ap):
            """psum -> sbuf bf16 with (optional bias add and) leaky relu."""
            if idx % DVE_RES_MOD < DVE_RES_LIM:
                nc.vector._custom_dve(
                    LEAKY_OP,
                    out=h,
                    in0=ph,
                    s0=0.0 if rb_ap is None else rb_ap,
                    s1=NEG,
                )
            else:
                if rb_ap is None:
                    nc.scalar.activation(h, ph, LRELU, alpha=NEG)
                else:
                    nc.scalar.activation(h, ph, LRELU, bias=rb_ap, alpha=NEG)

        res_idx = 0
        po_idx = 0
        pending_zsum = []  # deferred sum-over-k matmuls
        pending_tail = []  # deferred fused softmax-normalize
        pending_embmm = []  # emb matmul jpair closures
        pending_poev = []   # po eviction closures (lag one slot behind MMs)

        def emit(lst, n=99):
            for _ in range(min(n, len(lst))):
                lst.pop(0)()

        # prefetch x one chunk ahead on the gpsimd DMA queue
        chunks = [(s, c) for s in range(NSTACK) for c in range(2)]
        x_tiles = {}

        def fetch_x(k):
            if k >= len(chunks):
                return
            s_, c_ = chunks[k]
            t = xpool.tile([128, HB], dt.bfloat16, tag="x", name=f"x{s_}_{c_}")
            nc.gpsimd.dma_start(out=t, in_=xp[s_][:, c_ * HB : (c_ + 1) * HB])
            x_tiles[k] = t

        x_tiles[0] = xpool_first
        for s in range(NSTACK):
            e_sb = epool.tile([128, B], dt.bfloat16, tag="e", name=f"e{s}")
            en_sb = epool.tile([128, B], dt.bfloat16, tag="en", name=f"en{s}")
            for c in range(2):
                # ---- h pipeline: pairs interleaved layer-step-wise; the
                # previous chunk's softmax/emb work drips in between steps ----
                ck = s * 2 + c
                fetch_x(ck + 1)
                x_sb = x_tiles.pop(ck)
                hs = [x_sb] * 4
                for step in range(4):  # 0: L1 (row-tiled), 1-3: residual layers
                    for j in range(4):
                        p = 4 * s + j
                        ph = h_ps.tile(
                            [128, HB], dt.float32, tag="h", name=f"ph{p}_{c}_{step}"
                        )
                        if step == 0:
                            wsl = w1_sb[32 * j : 32 * j + 8, s * 128 : (s + 1) * 128]
                            rb_ap = None
                            for q in range(2):
                                nc.tensor.matmul(
                                    ph[:, q * 512 : (q + 1) * 512],
                                    wsl,
                                    hs[j][32 * j : 32 * j + 8, q * 512 : (q + 1) * 512],
                                    start=True,
                                    stop=True,
                                    tile_position=(32 * j, 0),
                                )
                        else:
                            l = step - 1
                            wsl = rw_sb[
                                :, (l * NPAIR + p) * 128 : (l * NPAIR + p + 1) * 128
                            ]
                            rb_ap = rb_sb[:, l * NPAIR + p : l * NPAIR + p + 1]
                            for q in range(2):
                                nc.tensor.matmul(
                                    ph[:, q * 512 : (q + 1) * 512],
                                    wsl,
                                    hs[j][:, q * 512 : (q + 1) * 512],
                                    start=True,
                                    stop=True,
                                )
                        h2 = hpool.tile(
                            [128, HB], dt.bfloat16, tag="h", name=f"h{p}_{c}_{step}"
                        )
                        evict_h(res_idx, h2, ph, rb_ap)
                        res_idx += 1
                        hs[j] = h2
                    # keep the PE array active across the eviction-paced gap
                    # so the HAM clock gate stays at full rate (LDWEIGHTS has
                    # no deps and touches no PSUM)
                    for _ in range(N_FILL):
                        nc.tensor.ldweights(weights=rw_sb[:, 0:128])
                    if step == 1:
                        emit(pending_zsum)
                    elif step == 2:
                        emit(pending_tail)
                    elif step == 3:
                        emit(pending_embmm, 1)
                pz = zs_ps.tile([128, HB], dt.float32, tag="zs", name=f"pz{s}_{c}")
                for j in range(4):
                    p = 4 * s + j
                    for q in range(2):
                        nc.tensor.matmul(
                            pz[32 * j : 32 * j + 32, q * 512 : (q + 1) * 512],
                            w2_sb[:, p * 32 : (p + 1) * 32],
                            hs[j][:, q * 512 : (q + 1) * 512],
                            start=True,
                            stop=True,
                            tile_position=(0, 32 * j),
                        )
                # interleave the previous chunk's emb MMs and (lagged) po
                # evictions so the ACT/DVE queues never park on an eviction
                # whose matmuls haven't run yet.
                while pending_embmm or pending_poev:
                    emit(pending_poev, 1)
                    if pending_embmm:
                        emit(pending_embmm, 1)

                def make_z(s_, c_, pz_ref, e_ref, en_ref):
                    def z_chain():
                        t1 = tpool.tile(
                            [128, HB], dt.float32, tag="zt", name=f"t1_{s_}_{c_}"
                        )
                        nc.scalar.activation(
                            t1, pz_ref, LRELU,
                            bias=b2_sb[:, s_ : s_ + 1], alpha=NEG,
                        )
                        ev = e_ref[:, c_ * HB : (c_ + 1) * HB]
                        nc.scalar.activation(
                            ev, t1, AF.Exp, scale=tau_sb[:, s_ : s_ + 1]
                        )

                        def zsum():
                            ps_sum = zs_ps.tile(
                                [128, HB], dt.float32, tag="zs", name=f"psum{s_}_{c_}"
                            )
                            for q in range(2):
                                nc.tensor.matmul(
                                    ps_sum[:, q * 512 : (q + 1) * 512],
                                    ones_sb,
                                    ev[:, q * 512 : (q + 1) * 512],
                                    start=True,
                                    stop=True,
                                )

                            def tail():
                                nc.vector._custom_dve(
                                    NORM_OP,
                                    out=en_ref[:, c_ * HB : (c_ + 1) * HB],
                                    in0=ps_sum,
                                    in1=ev,
                                    s0=NORM_C0,
                                    s1=NORM_C1,
                                )

                            pending_tail.append(tail)

                        pending_zsum.append(zsum)

                    return z_chain

                # this chunk's z epilogue (t1+exp) is emitted right away:
                # the ACT queue reaches it well after the pz matmuls drain.
                make_z(s, c, pz, e_sb, en_sb)()

                def make_jpair(s_, qb_, jp_, en_ref, ob_ref):
                    # two emb row-groups (j = 2*jp_, 2*jp_+1) interleaved
                    # i-major so each LDWEIGHTS hides under the other
                    # group's matmul; evictions lag one jpair behind the
                    # matmuls so ACT/DVE queues never park on them.
                    def emit_jpair_mms():
                        js = (2 * jp_, 2 * jp_ + 1)
                        pos = {}
                        for j_ in js:
                            pos[j_] = h_ps.tile(
                                [128, 4, 256],
                                dt.float32,
                                tag="h",
                                name=f"po{s_}_{qb_}_{j_}",
                            )
                        for i in range(4):
                            bc2 = qb_ * 4 + i
                            for j_ in js:
                                nc.tensor.matmul(
                                    pos[j_][:, i, :],
                                    en_ref[
                                        32 * j_ : 32 * j_ + 16,
                                        bc2 * 128 : (bc2 + 1) * 128,
                                    ],
                                    emb_sb[
                                        32 * j_ : 32 * j_ + 16,
                                        s_ * 256 : (s_ + 1) * 256,
                                    ],
                                    start=True,
                                    stop=True,
                                    tile_position=(32 * j_, 0),
                                )

                        def evict_pair():
                            nonlocal po_idx
                            for j_ in js:
                                if po_idx % 2 == 0:
                                    nc.vector.tensor_copy(ob_ref[:, :, j_, :], pos[j_])
                                else:
                                    nc.scalar.activation(
                                        ob_ref[:, :, j_, :], pos[j_], AF.Copy
                                    )
                                po_idx += 1
                            # DMA this jpair's half right away so the output
                            # transfer overlaps the other jpair's evictions
                            nc.sync.dma_start(
                                out=out_r[qb_, s_][:, :, js[0] : js[0] + 2, :],
                                in_=ob_ref[:, :, js[0] : js[0] + 2, :],
                            )

                        pending_poev.append(evict_pair)

                    return emit_jpair_mms

                for qb in (2 * c, 2 * c + 1):
                    ob = opool.tile(
                        [128, 4, 4, 256], dt.bfloat16, tag="o", name=f"ob{s}_{qb}"
                    )
                    for jp in range(2):
                        pending_embmm.append(make_jpair(s, qb, jp, en_sb, ob))
        for _ in range(6):
            nc.tensor.ldweights(weights=rw_sb[:, 0:128])
        emit(pending_zsum)
        for _ in range(6):
            nc.tensor.ldweights(weights=rw_sb[:, 0:128])
        emit(pending_tail)
        while pending_embmm or pending_poev:
            if pending_embmm:
                emit(pending_embmm, 1)
            emit(pending_poev, 1)

    nc.compile()
    return nc


def _host_pack(inputs):
    """Pack full f32 inputs into per-core bf16 device arrays."""
    x = np.ascontiguousarray(inputs["x"], dtype=np.float32)
    w1 = np.asarray(inputs["w1"], dtype=np.float32)
    b1 = np.asarray(inputs["b1"], dtype=np.float32)
    w2 = np.asarray(inputs["w2"], dtype=np.float32)
    b2 = np.asarray(inputs["b2"], dtype=np.float32)
    tau = np.asarray(inputs["tau"], dtype=np.float32)
    emb = np.asarray(inputs["emb"], dtype=np.float32)
    rws = [np.asarray(inputs[f"rw{l}"], dtype=np.float32) for l in range(3)]
    rbs = [np.asarray(inputs[f"rb{l}"], dtype=np.float32) for l in range(3)]

    eye = np.eye(D, dtype=np.float32)
    xT = np.concatenate([x, np.ones((B, F, 1), np.float32)], axis=2)
    xT = np.ascontiguousarray(xT.transpose(1, 2, 0))  # [F, 4, B]
    w1a = np.concatenate([w1, b1[:, None, :]], axis=1)  # [F, 4, D]

    in_maps = []
    for cidx in range(NCORES):
        f0 = cidx * FC
        xpk = np.zeros((NSTACK, 128, B), BF16)
        w1k = np.zeros((128, NSTACK, 128), BF16)
        rwk = np.zeros((128, 3, NPAIR, 128), BF16)
        rbk = np.zeros((128, 3, NPAIR), np.float32)
        w2k = np.zeros((128, NPAIR, 32), BF16)
        b2k = np.zeros((128, NSTACK), np.float32)
        tauk = np.zeros((128, NSTACK), np.float32)
        # garbage partitions keep tau=0 so exp(0)=1 stays finite
        embk = np.zeros((128, NSTACK, 256), BF16)
        for pr in range(NPAIR):
            fa, fb = f0 + 2 * pr, f0 + 2 * pr + 1
            s, jj = pr // 4, pr % 4
            xpk[s, 32 * jj : 32 * jj + 4] = xT[fa]
            xpk[s, 32 * jj + 4 : 32 * jj + 8] = xT[fb]
            w1k[32 * jj : 32 * jj + 4, s, 0:64] = w1a[fa]
            w1k[32 * jj + 4 : 32 * jj + 8, s, 64:128] = w1a[fb]
            for l in range(3):
                rwk[0:64, l, pr, 0:64] = rws[l][fa] + eye
                rwk[64:128, l, pr, 64:128] = rws[l][fb] + eye
                rbk[0:64, l, pr] = rbs[l][fa]
                rbk[64:128, l, pr] = rbs[l][fb]
            w2k[0:64, pr, 0:8] = w2[fa]
            w2k[64:128, pr, 8:16] = w2[fb]
            for fi, ff in ((0, fa), (1, fb)):
                rows = slice(32 * jj + 8 * fi, 32 * jj + 8 * fi + 8)
                b2k[rows, s] = b2[ff]
                tauk[rows, s] = tau[ff]
                embk[rows, s, 128 * fi : 128 * fi + 128] = emb[ff]
        # sum-over-k stationary with broadcast to all 128 rows; garbage
        # partitions duplicate the pair's second feature so values stay sane.
        ob = np.zeros((128, 128), BF16)
        for jj in range(4):
            for g in range(4):
                src = 32 * jj + 8 * min(g, 1)
                ob[src : src + 8, 32 * jj + 8 * g : 32 * jj + 8 * g + 8] = 1
        m = {
            "xp": xpk,
            "w1p": w1k.reshape(128, NSTACK * 128),
            "rwp": rwk.reshape(128, 3 * NPAIR * 128),
            "rbp": rbk.reshape(128, 3 * NPAIR),
            "w2p": w2k.reshape(128, NPAIR * 32),
            "b2s": b2k,
            "taus": tauk,
            "embs": embk.reshape(128, NSTACK * 256),
            "onesbd": ob,
        }
        in_maps.append(m)
    return in_maps


def _get_compiled():
    global _compiled
    if _compiled is None:
        _compiled = _build_bass()
    return _compiled


def run_on_hw(in_maps, trace=False):
    from concourse import bass_utils

    nc = _get_compiled()
    res = bass_utils.run_bass_kernel_spmd(
        nc, in_maps, core_ids=list(range(NCORES)), trace=trace
    )
    return res


def kernel(**inputs):
    in_maps = _host_pack(inputs)
    res = run_on_hw(in_maps, trace=False)
    outs = [np.asarray(res.results[c]["out"], dtype=np.float32) for c in range(NCORES)]
    return np.concatenate(outs, axis=1)


# revision 35
# speedup vs baseline: 1.0308x; 1.0019x over previous
"""AutoDisBucketEncoder Trainium2 kernel (8 NeuronCores, feature-sharded).

Math (per feature f, batch b):
  h = lrelu(x_aug @ w1_aug)            # bias folded via ones column
  h = lrelu(h @ (rw_l + I) + rb_l)     # x3, residual folded into weights
  z = lrelu(h @ w2 + b2)
  e = exp(z * tau)
  out = (e / sum_k e) @ emb

Layout: features sharded 32/core; each core packs 2 features per 128
partitions (block-diagonal weights), streams the full 2048 batch as the
matmul moving dim.  Softmax runs in [k, b] layout; the sum-over-k and its
broadcast back to 128 partitions are done by one ones-block matmul; the
normalize is one fused custom DVE op (mul by 1-NR reciprocal).  Step-0
matmuls are row-tiled 4-ways (contraction 8).  The embedding matmul uses
normalized probabilities as the stationary operand so its PSUM output
lands directly in [batch, emb] layout; emb matmuls are emitted j-pairwise
interleaved so LDWEIGHTS hides under the other row-group's matmul.
All activations use Prelu (parametric_relu) so Exp and the relu share one
ACT table set (no ACT_TABLE_LOAD swaps).
"""

import sys

sys.path.insert(0, "/opt/trn_rl_repo")

import numpy as np
import ml_dtypes
from contextlib import ExitStack

BF16 = ml_dtypes.bfloat16
B, F, D, K, E = 2048, 256, 64, 8, 128
NCORES = 8
FC = F // NCORES          # 32 features per core
NPAIR = FC // 2           # 16
NSTACK = NPAIR // 4       # 4 stacks of 4 pairs
NEG = 0.01                # leaky slope
HB = B // 2               # 1024 batch half-chunk (2 PSUM banks in f32)

# h-chunk evictions: idx % MOD < LIM -> DVE custom-op path, else ACT op
DVE_RES_MOD = 2
DVE_RES_LIM = 1

_compiled = None
SIM_SAFE = False  # substitute Relu for Prelu so CoreSim can execute
N_FILL = 6  # LDWEIGHTS fillers per step boundary (HAM keep-warm)


def _register_leaky_bias():
    import numpy as np
    from concourse.dve_spec import Spec, Src0, C0, C1, maxx, lower
    from concourse.dve_ops import (
        DveOp, DveOpSpec, OPS, CUSTOM_DVE_SPECS, _SUB_OPCODE_FOR_NAME,
        _CUSTOM_DVE_ROW_BASE, has_src1,
    )

    if "LEAKY_BIAS_ANT" in CUSTOM_DVE_SPECS:
        return next(o for o in OPS if o.name == "LEAKY_BIAS_ANT")
    spec = Spec(
        body=maxx(Src0 + C0, (Src0 + C0) * C1),
        reference=lambda in0, in1, s0, s1, imm2: np.maximum(
            in0 + s0, (in0 + s0) * s1
        ).astype(np.float32),
    )
    row = _CUSTOM_DVE_ROW_BASE + len(OPS)
    shas = {}
    for ver in ("v3", "v4"):
        uops = lower(spec, ver=ver)
        shas[ver] = DveOpSpec(
            name="LEAKY_BIAS_ANT", opcode=row, uops=uops, rd1_en=has_src1(spec)
        ).sha(ver)
    op = DveOp("LEAKY_BIAS_ANT", spec, subdim=False, uops_sha=shas)
    OPS.append(op)
    CUSTOM_DVE_SPECS[op.name] = spec
    _SUB_OPCODE_FOR_NAME[op.name] = row
    return op


def _register_softmax_norm():
    """out = Src1 * approx_recip(Src0), seed + one Newton step (~0.4% rel).

    Src0 = softmax denominator (fp32, PSUM), Src1 = exp values (bf16).
    Folds reciprocal + cast + multiply into one DVE pass.
    """
    import numpy as np
    from concourse.dve_spec import Spec, Src0, Src1, C0, C1, Bin, AluOp, lower
    from concourse.dve_ops import (
        DveOp, DveOpSpec, OPS, CUSTOM_DVE_SPECS, _SUB_OPCODE_FOR_NAME,
        _CUSTOM_DVE_ROW_BASE, has_src1,
    )

    if "SOFTMAX_NORM_ANT" in CUSTOM_DVE_SPECS:
        return next(o for o in OPS if o.name == "SOFTMAX_NORM_ANT")

    _not_x = Bin(AluOp.BITWISE_NOT, Src0, Src0)
    _y0 = _not_x * C0
    _y1 = _y0 * (C1 - Src0 * _y0)

    def _ref(in0, in1, s0, s1, imm2):
        not_x = (~in0.astype(np.float32).view(np.int32)).view(np.float32)
        y0 = not_x * s0
        y1 = y0 * (s1 - in0 * y0)
        return (in1.astype(np.float32) * y1).astype(np.float32)

    spec = Spec(body=Src1 * _y1, reference=_ref)
    row = _CUSTOM_DVE_ROW_BASE + len(OPS)
    shas = {}
    for ver in ("v3", "v4"):
        uops = lower(spec, ver=ver)
        shas[ver] = DveOpSpec(
            name="SOFTMAX_NORM_ANT", opcode=row, uops=uops, rd1_en=has_src1(spec)
        ).sha(ver)
    op = DveOp("SOFTMAX_NORM_ANT", spec, subdim=False, uops_sha=shas)
    OPS.append(op)
    CUSTOM_DVE_SPECS[op.name] = spec
    _SUB_OPCODE_FOR_NAME[op.name] = row
    return op


# Chebyshev seed constants shared with RECIPROCAL_APPROX_FAST; one NR pass.
NORM_C0 = -0.23549792
NORM_C1 = 2.0017324


def _build_bass():
    import concourse.bass as bass  # noqa: F401
    import concourse.mybir as mybir
    import concourse.tile as tile
    from concourse import bacc

    LEAKY_OP = _register_leaky_bias()
    NORM_OP = _register_softmax_norm()

    dt = mybir.dt
    AF = mybir.ActivationFunctionType
    LRELU = AF.Relu if SIM_SAFE else AF.Prelu

    nc = bacc.Bacc("TRN2", target_bir_lowering=False, debug=False)

    w1p = nc.dram_tensor("w1p", [128, NSTACK * 128], dt.bfloat16, kind="ExternalInput").ap()
    # x packed tight (8 real rows per pair); scattered to 32-row groups on load
    xp = nc.dram_tensor("xp", [NSTACK, 32, B], dt.bfloat16, kind="ExternalInput").ap()
    rwp = nc.dram_tensor("rwp", [128, 3 * NPAIR * 128], dt.bfloat16, kind="ExternalInput").ap()
    rbp = nc.dram_tensor("rbp", [128, 3 * NPAIR], dt.float32, kind="ExternalInput").ap()
    w2p = nc.dram_tensor("w2p", [128, NPAIR * 32], dt.bfloat16, kind="ExternalInput").ap()
    b2s = nc.dram_tensor("b2s", [128, NSTACK], dt.float32, kind="ExternalInput").ap()
    taus = nc.dram_tensor("taus", [128, NSTACK], dt.float32, kind="ExternalInput").ap()
    onesbd = nc.dram_tensor("onesbd", [128, 128], dt.bfloat16, kind="ExternalInput").ap()
    embs = nc.dram_tensor("embs", [128, NSTACK * 256], dt.bfloat16, kind="ExternalInput").ap()
    out = nc.dram_tensor("out", [B, FC * E], dt.bfloat16, kind="ExternalOutput").ap()

    with tile.TileContext(nc) as tc, ExitStack() as ctx:
        const = ctx.enter_context(tc.tile_pool(name="const", bufs=1))
        xpool = ctx.enter_context(tc.tile_pool(name="xpool", bufs=3))
        hpool = ctx.enter_context(tc.tile_pool(name="hpool", bufs=8))
        tpool = ctx.enter_context(tc.tile_pool(name="tpool", bufs=2))
        epool = ctx.enter_context(tc.tile_pool(name="epool", bufs=2))
        opool = ctx.enter_context(tc.tile_pool(name="opool", bufs=4))
        h_ps = ctx.enter_context(tc.tile_pool(name="h_ps", bufs=3, space="PSUM"))
        zs_ps = ctx.enter_context(tc.tile_pool(name="zs_ps", bufs=1, space="PSUM"))

        # ---- constants into SBUF (all host-side pre-transposed, contiguous).
        # Inputs ride the gpsimd DMA queue so they never wait behind the
        # output DMAs (which block on eviction sems on the sync queue). ----
        w1_sb = const.tile([128, NSTACK * 128], dt.bfloat16)
        nc.gpsimd.dma_start(out=w1_sb, in_=w1p)
        def dma_x(tile_, s_, c_):
            for j_ in range(4):
                nc.gpsimd.dma_start(
                    out=tile_[32 * j_ : 32 * j_ + 8, :],
                    in_=xp[s_][8 * j_ : 8 * j_ + 8, c_ * HB : (c_ + 1) * HB],
                )

        # first x chunk right behind w1 so chunk 0 starts ~immediately
        xpool_first = xpool.tile([128, HB], dt.bfloat16, tag="x", name="x0_0")
        dma_x(xpool_first, 0, 0)
        rb_sb = const.tile([128, 3 * NPAIR], dt.float32)
        nc.gpsimd.dma_start(out=rb_sb, in_=rbp)
        rw_sb = const.tile([128, 3 * NPAIR * 128], dt.bfloat16)
        for l in range(3):
            nc.gpsimd.dma_start(
                out=rw_sb[:, l * NPAIR * 128 : (l + 1) * NPAIR * 128],
                in_=rwp[:, l * NPAIR * 128 : (l + 1) * NPAIR * 128],
            )
        w2_sb = const.tile([128, NPAIR * 32], dt.bfloat16)
        nc.gpsimd.dma_start(out=w2_sb, in_=w2p)
        b2_sb = const.tile([128, NSTACK], dt.float32)
        nc.gpsimd.dma_start(out=b2_sb, in_=b2s)
        tau_sb = const.tile([128, NSTACK], dt.float32)
        nc.gpsimd.dma_start(out=tau_sb, in_=taus)
        ones_sb = const.tile([128, 128], dt.bfloat16)
        nc.gpsimd.dma_start(out=ones_sb, in_=onesbd)
        emb_sb = const.tile([128, NSTACK * 256], dt.bfloat16)
        nc.gpsimd.dma_start(out=emb_sb, in_=embs)

        # out[b, fc*E] viewed as [qb(4), s(4), p(128), i(4), j(4), e(256)]
        out_r = out.rearrange("(qb i p) (s j e) -> qb s p i j e", p=128, i=4, j=4, e=256)

        def evict_h(idx, h, ph, rb_ap):
            """psum -> sbuf bf16 with (optional bias add and) leaky relu."""
            if idx % DVE_RES_MOD < DVE_RES_LIM:
                nc.vector._custom_dve(
                    LEAKY_OP,
                    out=h,
                    in0=ph,
                    s0=0.0 if rb_ap is None else rb_ap,
                    s1=NEG,
                )
            else:
                if rb_ap is None:
                    nc.scalar.activation(h, ph, LRELU, alpha=NEG)
                else:
                    nc.scalar.activation(h, ph, LRELU, bias=rb_ap, alpha=NEG)

        res_idx = 0
        po_idx = 0
        pending_zsum = []  # deferred sum-over-k matmuls
        pending_tail = []  # deferred fused softmax-normalize
        pending_embmm = []  # emb matmul jpair closures
        pending_poev = []   # po eviction closures (lag one slot behind MMs)

        def emit(lst, n=99):
            for _ in range(min(n, len(lst))):
                lst.pop(0)()

        # prefetch x one chunk ahead on the gpsimd DMA queue
        chunks = [(s, c) for s in range(NSTACK) for c in range(2)]
        x_tiles = {}

        def fetch_x(k):
            if k >= len(chunks):
                return
            s_, c_ = chunks[k]
            t = xpool.tile([128, HB], dt.bfloat16, tag="x", name=f"x{s_}_{c_}")
            dma_x(t, s_, c_)
            x_tiles[k] = t

        x_tiles[0] = xpool_first
        for s in range(NSTACK):
            e_sb = epool.tile([128, B], dt.bfloat16, tag="e", name=f"e{s}")
            en_sb = epool.tile([128, B], dt.bfloat16, tag="en", name=f"en{s}")
            for c in range(2):
                # ---- h pipeline: pairs interleaved layer-step-wise; the
                # previous chunk's softmax/emb work drips in between steps ----
                ck = s * 2 + c
                fetch_x(ck + 1)
                x_sb = x_tiles.pop(ck)
                hs = [x_sb] * 4
                for step in range(4):  # 0: L1 (row-tiled), 1-3: residual layers
                    for j in range(4):
                        p = 4 * s + j
                        ph = h_ps.tile(
                            [128, HB], dt.float32, tag="h", name=f"ph{p}_{c}_{step}"
                        )
                        if step == 0:
                            wsl = w1_sb[32 * j : 32 * j + 8, s * 128 : (s + 1) * 128]
                            rb_ap = None
                            for q in range(2):
                                nc.tensor.matmul(
                                    ph[:, q * 512 : (q + 1) * 512],
                                    wsl,
                                    hs[j][32 * j : 32 * j + 8, q * 512 : (q + 1) * 512],
                                    start=True,
                                    stop=True,
                                    tile_position=(32 * j, 0),
                                )
                        else:
                            l = step - 1
                            wsl = rw_sb[
                                :, (l * NPAIR + p) * 128 : (l * NPAIR + p + 1) * 128
                            ]
                            rb_ap = rb_sb[:, l * NPAIR + p : l * NPAIR + p + 1]
                            for q in range(2):
                                nc.tensor.matmul(
                                    ph[:, q * 512 : (q + 1) * 512],
                                    wsl,
                                    hs[j][:, q * 512 : (q + 1) * 512],
                                    start=True,
                                    stop=True,
                                )
                        h2 = hpool.tile(
                            [128, HB], dt.bfloat16, tag="h", name=f"h{p}_{c}_{step}"
                        )
                        evict_h(res_idx, h2, ph, rb_ap)
                        res_idx += 1
                        hs[j] = h2
                    # keep the PE array active across the eviction-paced gap
                    # so the HAM clock gate stays at full rate (LDWEIGHTS has
                    # no deps and touches no PSUM)
                    for _ in range(N_FILL):
                        nc.tensor.ldweights(weights=rw_sb[:, 0:128])
                    if step == 1:
                        emit(pending_zsum)
                    elif step == 2:
                        emit(pending_tail)
                    elif step == 3:
                        emit(pending_embmm, 1)
                pz = zs_ps.tile([128, HB], dt.float32, tag="zs", name=f"pz{s}_{c}")
                for j in range(4):
                    p = 4 * s + j
                    for q in range(2):
                        nc.tensor.matmul(
                            pz[32 * j : 32 * j + 32, q * 512 : (q + 1) * 512],
                            w2_sb[:, p * 32 : (p + 1) * 32],
                            hs[j][:, q * 512 : (q + 1) * 512],
                            start=True,
                            stop=True,
                            tile_position=(0, 32 * j),
                        )
                # interleave the previous chunk's emb MMs and (lagged) po
                # evictions so the ACT/DVE queues never park on an eviction
                # whose matmuls haven't run yet.
                while pending_embmm or pending_poev:
                    emit(pending_poev, 1)
                    if pending_embmm:
                        emit(pending_embmm, 1)

                def make_z(s_, c_, pz_ref, e_ref, en_ref):
                    def z_chain():
                        t1 = tpool.tile(
                            [128, HB], dt.float32, tag="zt", name=f"t1_{s_}_{c_}"
                        )
                        nc.scalar.activation(
                            t1, pz_ref, LRELU,
                            bias=b2_sb[:, s_ : s_ + 1], alpha=NEG,
                        )
                        ev = e_ref[:, c_ * HB : (c_ + 1) * HB]
                        nc.scalar.activation(
                            ev, t1, AF.Exp, scale=tau_sb[:, s_ : s_ + 1]
                        )

                        def zsum():
                            ps_sum = zs_ps.tile(
                                [128, HB], dt.float32, tag="zs", name=f"psum{s_}_{c_}"
                            )
                            for q in range(2):
                                nc.tensor.matmul(
                                    ps_sum[:, q * 512 : (q + 1) * 512],
                                    ones_sb,
                                    ev[:, q * 512 : (q + 1) * 512],
                                    start=True,
                                    stop=True,
                                )

                            def tail():
                                nc.vector._custom_dve(
                                    NORM_OP,
                                    out=en_ref[:, c_ * HB : (c_ + 1) * HB],
                                    in0=ps_sum,
                                    in1=ev,
                                    s0=NORM_C0,
                                    s1=NORM_C1,
                                )

                            pending_tail.append(tail)

                        pending_zsum.append(zsum)

                    return z_chain

                # this chunk's z epilogue (t1+exp) is emitted right away:
                # the ACT queue reaches it well after the pz matmuls drain.
                make_z(s, c, pz, e_sb, en_sb)()

                def make_jpair(s_, qb_, jp_, en_ref, ob_ref):
                    # two emb row-groups (j = 2*jp_, 2*jp_+1) interleaved
                    # i-major so each LDWEIGHTS hides under the other
                    # group's matmul; evictions lag one jpair behind the
                    # matmuls so ACT/DVE queues never park on them.
                    def emit_jpair_mms():
                        js = (2 * jp_, 2 * jp_ + 1)
                        pos = {}
                        for j_ in js:
                            pos[j_] = h_ps.tile(
                                [128, 4, 256],
                                dt.float32,
                                tag="h",
                                name=f"po{s_}_{qb_}_{j_}",
                            )
                        for i in range(4):
                            bc2 = qb_ * 4 + i
                            for j_ in js:
                                nc.tensor.matmul(
                                    pos[j_][:, i, :],
                                    en_ref[
                                        32 * j_ : 32 * j_ + 16,
                                        bc2 * 128 : (bc2 + 1) * 128,
                                    ],
                                    emb_sb[
                                        32 * j_ : 32 * j_ + 16,
                                        s_ * 256 : (s_ + 1) * 256,
                                    ],
                                    start=True,
                                    stop=True,
                                    tile_position=(32 * j_, 0),
                                )

                        def evict_pair():
                            nonlocal po_idx
                            for j_ in js:
                                if po_idx % 2 == 0:
                                    nc.vector.tensor_copy(ob_ref[:, :, j_, :], pos[j_])
                                else:
                                    nc.scalar.activation(
                                        ob_ref[:, :, j_, :], pos[j_], AF.Copy
                                    )
                                po_idx += 1
                            # DMA this jpair's half right away so the output
                            # transfer overlaps the other jpair's evictions
                            nc.sync.dma_start(
                                out=out_r[qb_, s_][:, :, js[0] : js[0] + 2, :],
                                in_=ob_ref[:, :, js[0] : js[0] + 2, :],
                            )

                        pending_poev.append(evict_pair)

                    return emit_jpair_mms

                for qb in (2 * c, 2 * c + 1):
                    ob = opool.tile(
                        [128, 4, 4, 256], dt.bfloat16, tag="o", name=f"ob{s}_{qb}"
                    )
                    for jp in range(2):
                        pending_embmm.append(make_jpair(s, qb, jp, en_sb, ob))
        for _ in range(6):
            nc.tensor.ldweights(weights=rw_sb[:, 0:128])
        emit(pending_zsum)
        for _ in range(6):
            nc.tensor.ldweights(weights=rw_sb[:, 0:128])
        emit(pending_tail)
        while pending_embmm or pending_poev:
            if pending_embmm:
                emit(pending_embmm, 1)
            emit(pending_poev, 1)

    nc.compile()
    return nc


def _host_pack(inputs):
    """Pack full f32 inputs into per-core bf16 device arrays."""
    x = np.ascontiguousarray(inputs["x"], dtype=np.float32)
    w1 = np.asarray(inputs["w1"], dtype=np.float32)
    b1 = np.asarray(inputs["b1"], dtype=np.float32)
    w2 = np.asarray(inputs["w2"], dtype=np.float32)
    b2 = np.asarray(inputs["b2"], dtype=np.float32)
    tau = np.asarray(inputs["tau"], dtype=np.float32)
    emb = np.asarray(inputs["emb"], dtype=np.float32)
    rws = [np.asarray(inputs[f"rw{l}"], dtype=np.float32) for l in range(3)]
    rbs = [np.asarray(inputs[f"rb{l}"], dtype=np.float32) for l in range(3)]

    eye = np.eye(D, dtype=np.float32)
    xT = np.concatenate([x, np.ones((B, F, 1), np.float32)], axis=2)
    xT = np.ascontiguousarray(xT.transpose(1, 2, 0))  # [F, 4, B]
    w1a = np.concatenate([w1, b1[:, None, :]], axis=1)  # [F, 4, D]

    in_maps = []
    for cidx in range(NCORES):
        f0 = cidx * FC
        xpk = np.zeros((NSTACK, 32, B), BF16)
        w1k = np.zeros((128, NSTACK, 128), BF16)
        rwk = np.zeros((128, 3, NPAIR, 128), BF16)
        rbk = np.zeros((128, 3, NPAIR), np.float32)
        w2k = np.zeros((128, NPAIR, 32), BF16)
        b2k = np.zeros((128, NSTACK), np.float32)
        tauk = np.zeros((128, NSTACK), np.float32)
        # garbage partitions keep tau=0 so exp(0)=1 stays finite
        embk = np.zeros((128, NSTACK, 256), BF16)
        for pr in range(NPAIR):
            fa, fb = f0 + 2 * pr, f0 + 2 * pr + 1
            s, jj = pr // 4, pr % 4
            xpk[s, 8 * jj : 8 * jj + 4] = xT[fa]
            xpk[s, 8 * jj + 4 : 8 * jj + 8] = xT[fb]
            w1k[32 * jj : 32 * jj + 4, s, 0:64] = w1a[fa]
            w1k[32 * jj + 4 : 32 * jj + 8, s, 64:128] = w1a[fb]
            for l in range(3):
                rwk[0:64, l, pr, 0:64] = rws[l][fa] + eye
                rwk[64:128, l, pr, 64:128] = rws[l][fb] + eye
                rbk[0:64, l, pr] = rbs[l][fa]
                rbk[64:128, l, pr] = rbs[l][fb]
            w2k[0:64, pr, 0:8] = w2[fa]
            w2k[64:128, pr, 8:16] = w2[fb]
            for fi, ff in ((0, fa), (1, fb)):
                rows = slice(32 * jj + 8 * fi, 32 * jj + 8 * fi + 8)
                b2k[rows, s] = b2[ff]
                tauk[rows, s] = tau[ff]
                embk[rows, s, 128 * fi : 128 * fi + 128] = emb[ff]
        # sum-over-k stationary with broadcast to all 128 rows; garbage
        # partitions duplicate the pair's second feature so values stay sane.
        ob = np.zeros((128, 128), BF16)
        for jj in range(4):
            for g in range(4):
                src = 32 * jj + 8 * min(g, 1)
                ob[src : src + 8, 32 * jj + 8 * g : 32 * jj + 8 * g + 8] = 1
        m = {
            "xp": xpk,
            "w1p": w1k.reshape(128, NSTACK * 128),
            "rwp": rwk.reshape(128, 3 * NPAIR * 128),
            "rbp": rbk.reshape(128, 3 * NPAIR),
            "w2p": w2k.reshape(128, NPAIR * 32),
            "b2s": b2k,
            "taus": tauk,
            "embs": embk.reshape(128, NSTACK * 256),
            "onesbd": ob,
        }
        in_maps.append(m)
    return in_maps


def _get_compiled():
    global _compiled
    if _compiled is None:
        _compiled = _build_bass()
    return _compiled


def run_on_hw(in_maps, trace=False):
    from concourse import bass_utils

    nc = _get_compiled()
    res = bass_utils.run_bass_kernel_spmd(
        nc, in_maps, core_ids=list(range(NCORES)), trace=trace
    )
    return res


def kernel(**inputs):
    in_maps = _host_pack(inputs)
    res = run_on_hw(in_maps, trace=False)
    outs = [np.asarray(res.results[c]["out"], dtype=np.float32) for c in range(NCORES)]
    return np.concatenate(outs, axis=1)
